# revision 4
# baseline (speedup 1.0000x reference)
"""Trainium2 Bass kernel for nn_Attention_481036337444.

Dense single-layer attention: 1x1-conv QKV projection, 4 heads x 32 dims over
4096 pixels (64x64), softmax attention, 1x1-conv output projection.

Sharding: 16 (batch, head) pairs over 8 cores -> core c handles batch c//2 and
heads {2*(c%2), 2*(c%2)+1}. Host divides by the softmax denominators and sums
the per-core partial projections (+bias). No collectives.

The kernel is PSUM-evacuation bound: only ScalarE (ACT, ~(N+~80)/1.2 ns per
op) and VectorE (DVE, ~(N+120)/0.96 ns) can read PSUM, and evacuating
exp(sim) for the 2x4096x4096 logit matrix dominates (262144 columns/core).
Design (v2, from the 208.7us v1):
  - exp SPLIT across both engines: ACT does true exp via its LUT; DVE does
    a Schraudolph exp (tensor_scalar add -> int16 -> bitcast fp16, ~1.7%
    rms multiplicative noise on half the j-blocks; measured rel err 3.5e-3
    vs the 2e-2 gate). q is pre-scaled by 1024*log2(e)*SCALE so PSUM sims
    are already in the Schraudolph domain; ACT recovers exp(s) via its
    free per-instruction scale.
  - per half-step (ic, p, hf): 16 j-blocks as 6 bursts sized [3,3,3,3,2,2]
    alternating DVE/ACT, each burst = row-band-concurrent K=32 sim MMs
    into one of TWO 3-bank PSUM stage tiles (ring of 2). Larger engine
    ops (1536 cols) amortize the per-op overhead vs 4x(2+2) in v1.
  - q/k/v/po staging PSUM lives on the 2-slot "acc" ring: at any time only
    one attn chain is active, so the complementary slot serves the small
    512-col projections without stealing sim ring slots.
  - attn@v per (ic, head): even jb -> rows 0:33 (PE col 0), odd jb ->
    rows 64:97 (col 64), col-concurrent chains over all 32 jb of (ic,p);
    ones-row gives the denominator halves for free. acc evacuation is ONE
    97-row copy (junk rows 33:64 are finite from staging reuse and hit
    zero weights in the K=97 oproj matmul).
  - startup: x DMA split (x[:, :2048] + rest) with triggers on sync;
    wmisc trigger on the scalar engine (HWDGE) in parallel; ACT exp
    table-load fired first; q/k projections scheduled as data lands.
  - tail: the last half-step self-interleaves its own attn pairs (lag-1
    burst), dden DMA for p=0 fires 2 half-steps early, po DMA is one
    transfer per ic.
  - softmax division on HOST from the exported fp16 denominator halves.
Note: run-to-run HW time is bimodal (full clock vs ~2.0GHz P0 downclock);
identical NEFF, alternates on back-to-back runs.
"""

import numpy as np

F16 = np.float16
HEADS = 4
DIM_HEAD = 32
SCALE = DIM_HEAD ** -0.5
L2E = 1.4426950408889634
QSCALE = SCALE * 1024.0 * L2E       # PSUM sim = s * 1024*log2(e)
ACT_SCALE = 1.0 / (1024.0 * L2E)    # ACT: exp(scale * y) = exp(s)
SCHRAU_B = 15360.0 - 44.0           # i16 = y + B; bitcast fp16 ~ exp(s)
P = 128      # partitions == channels
N = 4096     # pixels = 64*64
CH = 512     # i-chunk width
NCH = N // CH
NCORES = 8
NETB = 3     # eT ring depth

# bursts per half-step: (jbl0, njb, engine). D = DVE schraudolph,
# A = ACT true exp. 8 jb on each engine per half-step.
BURSTS = [(0, 3, 'D'), (3, 3, 'A'), (6, 3, 'D'),
          (9, 3, 'A'), (12, 2, 'D'), (14, 2, 'A')]
# attn pairs (of the pending step) interleaved after burst bi
ATTN_PFX = [1, 2, 4, 5, 6, 8]          # prefix sums
ATTN_PFX_S1 = [0, 0, 2, 4, 6, 8]       # step 1: vT not ready before b2
# last step: own pairs emitted with ~1-burst lag (pair 7 in the flush)
OWN_AFTER = {1: (0,), 2: (1, 2), 3: (3,), 4: (4, 5), 5: (6,)}

_NC_CACHE = {}


def _build_nc():
    from concourse import bacc, mybir
    from concourse.tile import TileContext

    f32 = mybir.dt.float32
    f16 = mybir.dt.float16
    i16 = mybir.dt.int16
    EXP = mybir.ActivationFunctionType.Exp

    nc = bacc.Bacc()
    x_ext = nc.declare_dram_parameter("x", [P, N], f16, isOutput=False)
    # wmisc cols: [0:256] wq_rep(x QSCALE), [256:320] wk_t, [320:384] wv_t,
    # [384:512] wo_h0 (rows 0:33 and 64:97), [512:640] wo_h1
    wm_ext = nc.declare_dram_parameter("wmisc", [P, 640], f16, isOutput=False)
    # po cols: ic*1024 + 512*p : unnormalized per-head projected outputs
    po_ext = nc.declare_dram_parameter("po", [P, 2 * N], f16, isOutput=True)
    # denominator halves: rows (2*p + half) = headout row {32,96} of head p
    dd_ext = nc.declare_dram_parameter("dden", [4, N], f16, isOutput=True)

    with TileContext(nc) as tc:
        with (
            tc.tile_pool(name="persist", bufs=1) as persist,
            tc.tile_pool(name="sbB", bufs=2) as sbB,
            tc.tile_pool(name="ps", space="PSUM", bufs=2) as ps,
        ):
            # ---- DMA triggers: x halves on sync, wmisc on scalar (HWDGE)
            xt = persist.tile([P, N], f16)
            wmisc = persist.tile([P, 640], f16)
            nc.sync.dma_start(out=xt[:, 0:N // 2], in_=x_ext[:, 0:N // 2])
            nc.scalar.dma_start(out=wmisc[:], in_=wm_ext[:])
            nc.sync.dma_start(out=xt[:, N // 2:], in_=x_ext[:, N // 2:])

            # ---- ACT exp table load off the critical path
            wup = persist.tile([P, 8], f32)
            nc.vector.memset(wup[:], 0.0)
            wup2 = persist.tile([P, 8], f32)
            nc.scalar.activation(out=wup2[:], in_=wup[:], func=EXP)

            wq_rep = wmisc[:, 0:256]
            wk_t = wmisc[:, 256:320]
            wv_t = wmisc[:, 320:384]

            # q4m: [128, 8192]: col = 1024*ic + 512*p + i  (d-band part)
            q4m = persist.tile([P, 2 * N], f16)
            # k4m: [128, 2048]: col = 1024*hf + 512*p + 128*(jbl//4) + j
            k4m = persist.tile([P, 2048], f16)
            # vTm: [128, 32 jb, 66]: per jb, cols 0:33 = head0 [v^T | 1],
            # cols 33:66 = head1
            vTm = persist.tile([P, 32 * 66], f16)
            vTr = vTm.rearrange("a (j m) -> a j m", m=66)
            nc.vector.memset(vTr[:, :, 32:33], 1.0)
            nc.vector.memset(vTr[:, :, 65:66], 1.0)
            vTp = vTm.rearrange("a (j p m) -> a j p m", p=2, m=33)
            # headout: cols 4096*p + ic*512; rows 0:33 even-half (+den@32),
            # rows 64:97 odd-half (+den@96). rows 33:64, 97:128 unused.
            headout = persist.tile([P, 2 * N], f16)
            eT = [persist.tile([P, 16 * CH], f16, name=f"expT{h}")
                  for h in range(NETB)]

            # x columns as [b(2), u(4), t(4), j(128)]: col = 2048b+512u+128t+j
            xr = xt.rearrange("c (b u t j) -> c b u t j", b=2, u=4, t=4, j=128)

            def emit_q(ic, p, eng='A'):
                pq = ps.tile([P, CH], f32, tag="acc", name="pq")
                nc.tensor.matmul(
                    out=pq[:, 0:CH],
                    lhsT=wq_rep[:, p * 128:(p + 1) * 128],
                    rhs=xt[:, ic * CH:(ic + 1) * CH],
                    tile_position=(0, 0),
                )
                dst = q4m[:, 1024 * ic + 512 * p:1024 * ic + 512 * (p + 1)]
                if eng == 'A':
                    nc.scalar.copy(dst, pq[:, 0:CH])
                else:
                    nc.vector.tensor_copy(dst, pq[:, 0:CH])

            def emit_k(hf, p, eng='D'):
                pk = ps.tile([P, CH], f32, tag="acc", name="pk")
                for t in range(4):
                    nc.tensor.matmul(
                        out=pk[32 * t:32 * t + 32, 0:CH],
                        lhsT=wk_t[:, 32 * p:32 * p + 32],
                        rhs=xr[:, hf, :, t, :],
                        tile_position=(0, 32 * t),
                    )
                dst = k4m[:, 1024 * hf + 512 * p:1024 * hf + 512 * (p + 1)]
                if eng == 'A':
                    nc.scalar.copy(dst, pk[:, 0:CH])
                else:
                    nc.vector.tensor_copy(dst, pk[:, 0:CH])

            def emit_v(g):
                pv = ps.tile([P, CH], f32, tag="acc", name="pv")
                for j in range(8):
                    jb = 8 * g + j
                    nc.tensor.matmul(
                        out=pv[:, 64 * j:64 * j + 64],
                        lhsT=xt[:, 128 * jb:128 * jb + 128],
                        rhs=wv_t[:],
                        tile_position=(0, 0),
                    )
                pvr = pv[:, 0:CH].rearrange("a (j p m) -> a j p m",
                                            p=2, m=32)
                nc.vector.tensor_copy(vTp[:, 8 * g:8 * g + 8, :, 0:32], pvr)

            accs = {}

            def emit_burst(s, ic, p, hf, bi):
                jbl0, njb, eng = BURSTS[bi]
                sg = ps.tile([P, 3 * CH], f32, tag="stage", name="sg")
                for k in range(njb):
                    jbl = jbl0 + k
                    jb = 16 * hf + jbl
                    t = jbl % 4
                    u4 = jbl // 4
                    nc.tensor.matmul(
                        out=sg[:, k * CH:(k + 1) * CH],
                        lhsT=k4m[32 * t:32 * t + 32,
                                 1024 * hf + 512 * p + 128 * u4:
                                 1024 * hf + 512 * p + 128 * u4 + 128],
                        rhs=q4m[32 * t:32 * t + 32,
                                1024 * ic + 512 * p:
                                1024 * ic + 512 * (p + 1)],
                        tile_position=(32 * t, 0),
                    )
                buf = eT[s % NETB]
                if eng == 'D':
                    nc.vector.tensor_scalar_add(
                        buf[:, jbl0 * CH:(jbl0 + njb) * CH].bitcast(i16),
                        sg[:, 0:njb * CH], SCHRAU_B)
                else:
                    nc.scalar.activation(
                        out=buf[:, jbl0 * CH:(jbl0 + njb) * CH],
                        in_=sg[:, 0:njb * CH], func=EXP, scale=ACT_SCALE)

            def emit_attn_pairs(s, ic, p, hf, pairs):
                if not pairs:
                    return
                buf = eT[s % NETB]
                if hf == 0 and pairs[0] == 0:
                    accs[p] = ps.tile([P, CH], f32, tag="acc", name="acc")
                acc = accs[p]
                for pr in pairs:
                    for jbl in (2 * pr, 2 * pr + 1):
                        jb = 16 * hf + jbl
                        col = 0 if jb % 2 == 0 else 64
                        nc.tensor.matmul(
                            out=acc[col:col + 33, :],
                            lhsT=vTp[:, jb, p, :],
                            rhs=buf[:, jbl * CH:(jbl + 1) * CH],
                            tile_position=(0, col),
                            start=(jb < 2),
                            stop=(jb >= 30),
                            skip_group_check=True,
                        )

            pend_oproj = []
            pend_po = []
            nacc = [0]

            def emit_acc_evac(ic, p):
                # after (ic, p)'s chains complete: evacuate raw acc (+den
                # rows) to headout with one 97-row copy; junk rows 33:64
                # carry finite staging leftovers that hit zero weights in
                # the K=97 oproj matmul. Engine split 11 ACT / 5 DVE for
                # balance.
                acc = accs[p]
                hcol = N * p + ic * CH
                if nacc[0] % 3 == 1:
                    nc.vector.tensor_copy(headout[0:97, hcol:hcol + CH],
                                          acc[0:97])
                else:
                    nc.scalar.copy(headout[0:97, hcol:hcol + CH], acc[0:97])
                nacc[0] += 1
                if p == 1:
                    pend_oproj.append(ic)

            def emit_oproj():
                while pend_oproj:
                    ic = pend_oproj.pop(0)
                    pos = []
                    for p in range(2):
                        hcol = N * p + ic * CH
                        po = ps.tile([P, CH], f32, tag="acc", name="po")
                        wo = wmisc[:, 384 + 128 * p:512 + 128 * p]
                        nc.tensor.matmul(
                            out=po[:, 0:CH], lhsT=wo[0:97, :],
                            rhs=headout[0:97, hcol:hcol + CH],
                            tile_position=(0, 0),
                        )
                        pos.append(po)
                    pend_po.append((ic, pos))

            def emit_po_evac():
                while pend_po:
                    ic, pos = pend_po.pop(0)
                    ob = sbB.tile([P, 2 * CH], f16, tag="outbuf", name="ob")
                    for p in range(2):
                        nc.scalar.copy(ob[:, p * CH:(p + 1) * CH],
                                       pos[p][:, 0:CH])
                    nc.sync.dma_start(
                        out=po_ext[:, ic * 1024:(ic + 1) * 1024],
                        in_=ob[:],
                    )

            def emit_dden(p):
                for half, row in ((0, 32), (1, 96)):
                    nc.sync.dma_start(
                        out=dd_ext[2 * p + half:2 * p + half + 1, :],
                        in_=headout[row:row + 1, N * p:N * (p + 1)],
                    )

            # ---- startup: q and k(hf=0) as soon as x[:, :2048] lands
            emit_q(0, 0)
            emit_q(0, 1)
            emit_k(0, 0)

            steps = [(ic, p, hf) for ic in range(NCH) for p in range(2)
                     for hf in range(2)]
            S = len(steps)
            pending = None
            for s, (ic, p, hf) in enumerate(steps):
                pfx = ATTN_PFX_S1 if s == 1 else ATTN_PFX
                for bi in range(6):
                    emit_burst(s, ic, p, hf, bi)
                    if pending is not None:
                        lo = 0 if bi == 0 else pfx[bi - 1]
                        emit_attn_pairs(pending[3], *pending[:3],
                                        tuple(range(lo, pfx[bi])))
                    if s == S - 1 and bi in OWN_AFTER:
                        emit_attn_pairs(s, ic, p, hf, OWN_AFTER[bi])
                    if s == 0:
                        if bi == 0:
                            emit_k(0, 1, eng='A')
                        elif bi == 1:
                            emit_k(1, 0, eng='A')
                        elif bi == 2:
                            emit_k(1, 1, eng='A')
                        elif bi == 3:
                            emit_v(0)
                        elif bi == 4:
                            emit_v(1)
                        elif bi == 5:
                            emit_v(2)
                    elif s == 1:
                        if bi == 1:
                            emit_v(3)
                    else:
                        if bi == 1:
                            emit_oproj()
                        elif bi == 2 and p == 1 and hf == 0 and ic + 1 < NCH:
                            emit_q(ic + 1, 0)
                        elif bi == 3 and p == 1 and hf == 0 and ic + 1 < NCH:
                            emit_q(ic + 1, 1)
                        elif bi == 4:
                            emit_po_evac()
                if pending is not None and pending[2] == 1:
                    emit_acc_evac(pending[0], pending[1])
                    if pending[0] == NCH - 1 and pending[1] == 0:
                        emit_dden(0)
                pending = (ic, p, hf, s)
            # ---- flush
            emit_attn_pairs(S - 1, NCH - 1, 1, 1, (7,))
            emit_acc_evac(NCH - 1, 1)
            emit_oproj()
            emit_po_evac()
            emit_dden(1)

    nc.finalize()
    return nc


def _get_nc():
    if "nc" not in _NC_CACHE:
        _NC_CACHE["nc"] = _build_nc()
    return _NC_CACHE["nc"]


def _prep_core(x, w_qkv, w_out, c):
    b, s = divmod(c, 2)
    h0 = 2 * s
    xc = np.ascontiguousarray(x[b].reshape(P, N)).astype(F16)
    wmisc = np.zeros((P, 640), np.float32)
    for p in range(2):
        h = h0 + p
        wq = w_qkv[32 * h:32 * h + 32, :]
        wk = w_qkv[128 + 32 * h:128 + 32 * h + 32, :]
        wv = w_qkv[256 + 32 * h:256 + 32 * h + 32, :]
        wmisc[:, 128 * p:128 * (p + 1)] = np.tile(
            (wq.T * QSCALE).astype(np.float32), (1, 4))
        wmisc[:, 256 + 32 * p:256 + 32 * (p + 1)] = wk.T
        wmisc[:, 320 + 32 * p:320 + 32 * (p + 1)] = wv.T
        # wo for head p, rows 0:32 and 64:96 (merges even/odd chain halves)
        wo = w_out[:, 32 * h:32 * h + 32].T     # [32, 128]
        wmisc[0:32, 384 + 128 * p:512 + 128 * p] = wo
        wmisc[64:96, 384 + 128 * p:512 + 128 * p] = wo
    return {"x": xc, "wmisc": wmisc.astype(F16)}


def _run(in_maps, trace=False):
    from concourse.bass_utils import run_bass_kernel_spmd
    nc = _get_nc()
    return run_bass_kernel_spmd(nc, in_maps, core_ids=list(range(NCORES)),
                                trace=trace)


def kernel(**inputs):
    x = np.asarray(inputs["x"], np.float32)
    w_qkv = np.asarray(inputs["w_qkv"], np.float32)
    w_out = np.asarray(inputs["w_out"], np.float32)
    b_out = np.asarray(inputs["b_out"], np.float32)

    in_maps = [_prep_core(x, w_qkv, w_out, c) for c in range(NCORES)]
    res = _run(in_maps)
    B = x.shape[0]
    out = np.empty((B, P, 64, 64), np.float32)
    for b in range(B):
        o = np.zeros((P, N), np.float64)
        for s in range(2):
            r = res.results[2 * b + s]
            po = np.asarray(r["po"], np.float32).reshape(P, NCH, 2, CH)
            dd = np.asarray(r["dden"], np.float32)
            for p in range(2):
                den = (dd[2 * p] + dd[2 * p + 1]).reshape(NCH, CH)
                o += (po[:, :, p, :] / den[None, :, :]).reshape(P, N)
        o = o + b_out[:, None]
        out[b] = o.reshape(P, 64, 64).astype(np.float32)
    return out
